# revision 8
# baseline (speedup 1.0000x reference)
"""Single-head attention (B=4, S=4096, E=1024, D=64) on 8 TRN2 NeuronCores.

Sharding: data-parallel over (batch, query-half): core c handles batch
b = c // 2 and query rows [h*2048, (h+1)*2048) with h = c % 2. Each core
computes Q for its own 2048 rows and K/V for the full 4096 rows of its batch
(inputs are shipped host-pretransposed per half, so no duplicated DMA).

Per-core dataflow (TensorE matmuls in bf16 — fp32/fp32r matmuls run the PE
at half clock; fp32 accumulation in PSUM):
  x^T (bf16)  --WqT/WkT/WvT-->  Q^T [64, 2048], K^T [64, S] (bf16),
  V^T_aug [65, S] (bf16, row 64 = ones) -> PE transpose -> V_aug [k, 65].
  scores^T[k, q] = K^T.T @ Q^T -> exp on ScalarE (scale folded) -> P bf16
  attn^T[65, q] += V_aug.T @ P   (row 64 accumulates softmax denominators)
  output = attn^T with denominators; host transposes + normalizes.

Emission is software-pipelined: K/V projection chunks and V transposes are
interleaved into the attention loop so ScalarE (exp) starts ~10us in and the
PE never waits on a phase boundary.
"""

import numpy as np

B, S, E, D = 4, 4096, 1024, 64
HALF = S // 2
N_CORES = 8
SCALE = 1.0 / np.sqrt(D)

NE = E // 128  # 8 e-tiles
NKT = S // 128  # 32 k-tiles
NCH = S // 512  # 8 proj column-chunks of 512 for K/V

_CACHE = {}


def _build():
    if "nc" in _CACHE:
        return _CACHE["nc"]

    from contextlib import ExitStack

    import concourse.bacc as bacc
    import concourse.tile as tile
    from concourse import mybir
    from concourse.masks import make_identity

    FP32 = mybir.dt.float32
    BF16 = mybir.dt.bfloat16
    Exp = mybir.ActivationFunctionType.Exp

    nc = bacc.Bacc(
        "TRN2", target_bir_lowering=False, debug=False, num_devices=N_CORES
    )

    xt_q_d = nc.dram_tensor("xt_q", [E, HALF], BF16, kind="ExternalInput").ap()
    xt_o_d = nc.dram_tensor("xt_o", [E, HALF], BF16, kind="ExternalInput").ap()
    wt_d = nc.dram_tensor("wt", [E, 3 * D], BF16, kind="ExternalInput").ap()
    out_d = nc.dram_tensor("out", [D + 1, HALF], FP32, kind="ExternalOutput").ap()

    with tile.TileContext(nc) as tc, ExitStack() as ctx:
        const = ctx.enter_context(tc.tile_pool(name="const", bufs=1))
        big = ctx.enter_context(tc.tile_pool(name="big", bufs=1))
        pp = ctx.enter_context(tc.tile_pool(name="pp", bufs=6))
        psA = ctx.enter_context(tc.tile_pool(name="psA", bufs=2, space="PSUM"))
        psB = ctx.enter_context(tc.tile_pool(name="psB", bufs=1, space="PSUM"))

        identB = const.tile([128, 128], BF16)
        make_identity(nc, identB)

        xt = big.tile([128, NE, S], BF16)  # x^T; cols [0, HALF) = own q-rows
        wt = big.tile([128, NE, 3 * D], BF16)  # WqT | WkT | WvT
        qt = big.tile([64, HALF], BF16)  # Q^T
        kt = big.tile([64, S], BF16)  # K^T
        vt = big.tile([65, S], BF16)  # V^T staging; row 64 = ones
        vn = big.tile([128, NKT, D + 1], BF16)  # V natural + ones column
        att_sb = big.tile([65, HALF], FP32)  # attn^T + denominator row

        # --- input DMAs: wt first (Q projection gate), then x^T pieces.
        # Own half in 512-col pieces alternating sync/gpsimd for fast start.
        nc.sync.dma_start(out=wt[:, :, :], in_=wt_d.rearrange("(t p) d -> p t d", p=128))
        xq = xt_q_d.rearrange("(t p) s -> p t s", p=128)
        xo = xt_o_d.rearrange("(t p) s -> p t s", p=128)
        for c in range(4):
            eng = nc.gpsimd if c % 2 == 0 else nc.sync
            eng.dma_start(
                out=xt[:, :, c * 512 : (c + 1) * 512],
                in_=xq[:, :, c * 512 : (c + 1) * 512],
            )
        for c in range(4):
            eng = nc.gpsimd if c % 2 == 0 else nc.sync
            eng.dma_start(
                out=xt[:, :, HALF + c * 512 : HALF + (c + 1) * 512],
                in_=xo[:, :, c * 512 : (c + 1) * 512],
            )

        nc.vector.memset(vt[64:65, :], 1.0)

        # one projection column-chunk of 512: dst[:, cols] = (x W_widx)^T
        def proj_chunk(widx, dst, c):
            acc = psA.tile([128, 1024], FP32, tag="ps")
            for et in range(NE):
                nc.tensor.matmul(
                    out=acc[0:64, 0:512],
                    lhsT=wt[:, et, widx * D : (widx + 1) * D],
                    rhs=xt[:, et, c * 512 : (c + 1) * 512],
                    start=(et == 0),
                    stop=(et == NE - 1),
                )
            nc.vector.tensor_copy(
                out=dst[:, c * 512 : (c + 1) * 512], in_=acc[0:64, 0:512]
            )

        def v_transpose(k):
            tp = psA.tile([128, 1024], BF16, tag="ps")
            nc.tensor.transpose(
                out=tp[0:128, 0:65],
                in_=vt[:, k * 128 : (k + 1) * 128],
                identity=identB[0:65, 0:65],
            )
            nc.vector.tensor_copy(out=vn[:, k, :], in_=tp[0:128, 0:65])

        # --- prologue: Q fully; K/V chunks 0-1; V transposes 0-3 ---
        for c in range(HALF // 512):
            proj_chunk(0, qt, c)
        for c in range(2):
            proj_chunk(1, kt, c)
            proj_chunk(2, vt[0:64, :], c)
        for k in range(4):
            v_transpose(k)

        att_ps = psB.tile([128, HALF], FP32)
        p_tiles = {}

        # --- pipelined attention loop ---
        for k in range(NKT):
            if k % 4 == 0 and k // 4 + 2 < NCH:
                proj_chunk(1, kt, k // 4 + 2)
                proj_chunk(2, vt[0:64, :], k // 4 + 2)
            if k + 4 < NKT:
                v_transpose(k + 4)

            for h in range(2):
                sc = psA.tile([128, 1024], FP32, tag="ps")
                for c in range(2):
                    q0 = h * 1024 + c * 512
                    nc.tensor.matmul(
                        out=sc[:, c * 512 : (c + 1) * 512],
                        lhsT=kt[:, k * 128 : (k + 1) * 128],
                        rhs=qt[:, q0 : q0 + 512],
                        start=True,
                        stop=True,
                    )
                p = pp.tile([128, 1024], BF16)
                nc.scalar.activation(out=p[:, :], in_=sc[:, :], func=Exp, scale=SCALE)
                p_tiles[(k, h)] = p

            if k >= 2:
                _attn(nc, att_ps, vn, p_tiles, k - 2)

        _attn(nc, att_ps, vn, p_tiles, NKT - 2)
        _attn(nc, att_ps, vn, p_tiles, NKT - 1)

        # --- ship attn^T + denominators; host transposes + normalizes ---
        nc.vector.tensor_copy(out=att_sb[:, :], in_=att_ps[0:65, :])
        nc.sync.dma_start(out=out_d[:, :], in_=att_sb[:, :])

    nc.compile()
    _CACHE["nc"] = nc
    return nc


def _attn(nc, att_ps, vn, p_tiles, k):
    for h in range(2):
        p = p_tiles.pop((k, h))
        for c in range(2):
            q0 = h * 1024 + c * 512
            nc.tensor.matmul(
                out=att_ps[0:65, q0 : q0 + 512],
                lhsT=vn[:, k, :],
                rhs=p[:, c * 512 : (c + 1) * 512],
                start=(k == 0),
                stop=(k == NKT - 1),
                skip_group_check=True,
            )


def _make_in_maps(x, Wq, Wk, Wv):
    import ml_dtypes

    bf16 = ml_dtypes.bfloat16
    xT = np.ascontiguousarray(x.transpose(0, 2, 1)).astype(bf16)  # [B, E, S]
    wt = np.concatenate([Wq.T, Wk.T, Wv.T], axis=1).astype(bf16)  # [E, 3D]
    in_maps = []
    for c in range(N_CORES):
        b, h = divmod(c, 2)
        in_maps.append(
            {
                "xt_q": np.ascontiguousarray(xT[b, :, h * HALF : (h + 1) * HALF]),
                "xt_o": np.ascontiguousarray(
                    xT[b, :, (1 - h) * HALF : (2 - h) * HALF]
                ),
                "wt": wt,
            }
        )
    return in_maps


def _run(x, Wq, Wk, Wv, trace=False):
    from concourse.bass_utils import run_bass_kernel_spmd

    nc = _build()
    in_maps = _make_in_maps(x, Wq, Wk, Wv)
    res = run_bass_kernel_spmd(
        nc, in_maps, core_ids=list(range(N_CORES)), trace=trace
    )
    out = np.empty((B, S, D), dtype=np.float32)
    for c in range(N_CORES):
        b, h = divmod(c, 2)
        att = res.results[c]["out"]  # [65, HALF]: attn^T rows + denom row
        out[b, h * HALF : (h + 1) * HALF, :] = (att[0:D] / att[D : D + 1]).T
    return out, res


def kernel(x, Wq, Wk, Wv):
    out, _ = _run(
        np.asarray(x, dtype=np.float32),
        np.asarray(Wq, dtype=np.float32),
        np.asarray(Wk, dtype=np.float32),
        np.asarray(Wv, dtype=np.float32),
    )
    return out
